# revision 29
# baseline (speedup 1.0000x reference)
"""Fused transformer block (pre-norm attention + MLP) for Trainium2, 8 NeuronCores.

Problem: B=2, S=2048, D=512, H=8, HD=64, fp32 in/out.

Sharding: pure data-parallel over (batch, query-block). Core c handles batch
b = c//4 and query rows [qb*512, (qb+1)*512) with qb = c%4. Each core
redundantly computes LN1 + K/V for the full 2048-token sequence of its batch
element (no collectives needed).

v2 changes vs baseline:
- bf16 datapath for every matmul (weights pre-cast on host; activations
  written bf16 at PSUM evacuation). PSUM accumulation stays fp32. Residual
  stream (xto/xat/yt) stays fp32.
- K bias dropped entirely (constant-per-query score shift cancels in
  softmax); V bias folded into the proj bias on host (bp' = bp + bv@Wp).
- The redundant own-block re-normalization (xno) removed; Q projects from
  the already-normalized xtf slice.
- MLP ReLU+bias moved to ScalarE (Relu is in the natural_log_exp table set,
  so no table reload); frees the Vector engine.
- LN math in bf16 2x DVE mode where operands live in SBUF.
- LN2 stats use f32r bitcast views of the fp32 residual (no cast pass).
"""

import os
import sys

sys.path.insert(0, "/opt/trn_rl_repo")

import numpy as np
import ml_dtypes

import concourse.bass as bass
import concourse.mybir as mybir
import concourse.tile as tile
from concourse import bacc
from concourse.bass_utils import run_bass_kernel_spmd

P = 128
B, S, D, H = 2, 2048, 512, 8
HD = D // H          # 64
DFF = 4 * D          # 2048
TO = 512             # tokens owned per core (query block)
NC = D // P          # 4 d-chunks
NE = D // P          # 4 e_all-subs (q/k head pairs)
NT = S // P          # 16 t-subs
NN = DFF // P        # 16 n-subs
EPS = 1e-5
F32 = mybir.dt.float32
F32R = mybir.dt.float32r
BF16 = mybir.dt.bfloat16
AF = mybir.ActivationFunctionType
OP = mybir.AluOpType
BF = ml_dtypes.bfloat16
FP16 = mybir.dt.float16
F16 = np.float16

LAST_EXEC_NS = None

# Force every Exp/Ln ACTIVATE onto the one table set containing both, so the
# ACT table never reloads mid-kernel (each reload is ~2.7us and serializes
# against the attention exps). Copy/Identity/Relu/Square are in every set.
_orig_gat = bacc.get_activation_tables


def _gat_nlexp_only(arch):
    tabs = _orig_gat(arch)
    for name, fns in tabs.items():
        if name != "natural_log_exp_and_others":
            fns.discard(AF.Exp)
            fns.discard(AF.Ln)
    return tabs


bacc.get_activation_tables = _gat_nlexp_only

def build_program():
    nc = bacc.Bacc("TRN2", target_bir_lowering=False, debug=False, num_devices=8)

    xtf_d = nc.dram_tensor("xtf", [P, NC, S], FP16, kind="ExternalInput")
    xto_d = nc.dram_tensor("xto", [P, NC, TO], F32, kind="ExternalInput")
    wq_d = nc.dram_tensor("wq", [P, NC, D], FP16, kind="ExternalInput")
    wk_d = nc.dram_tensor("wk", [P, NC, D], FP16, kind="ExternalInput")
    wv_d = nc.dram_tensor("wv", [P, NC, D], FP16, kind="ExternalInput")
    wp_d = nc.dram_tensor("wp", [P, NC, D], FP16, kind="ExternalInput")
    w1_d = nc.dram_tensor("w1", [P, NC, DFF], FP16, kind="ExternalInput")
    w2_d = nc.dram_tensor("w2", [P, NN, D], FP16, kind="ExternalInput")
    bq_d = nc.dram_tensor("bq", [P, NE], F32, kind="ExternalInput")
    bp_d = nc.dram_tensor("bp", [P, NC], F32, kind="ExternalInput")
    b1_d = nc.dram_tensor("b1", [P, NN], F32, kind="ExternalInput")
    b2_d = nc.dram_tensor("b2", [P, NC], F32, kind="ExternalInput")
    onesh_d = nc.dram_tensor("onesh", [P, P], FP16, kind="ExternalInput")
    zer_d = nc.dram_tensor("zer64", [HD, TO], FP16, kind="ExternalInput")
    ones1_d = nc.dram_tensor("ones1", [1, P], BF16, kind="ExternalInput")
    yt_d = nc.dram_tensor("yt", [P, NC, TO], F32, kind="ExternalOutput")

    with tile.TileContext(nc) as tc:
        with (
            tc.tile_pool(name="main", bufs=1) as mp,
            tc.tile_pool(name="db", bufs=2) as db,
            tc.tile_pool(name="expp", bufs=3) as expp,
            tc.tile_pool(name="pbig", bufs=3, space="PSUM") as pbig,
            tc.tile_pool(name="po", bufs=2, space="PSUM") as po,
        ):
            # ---- constants / weights (all prefetched up front) ----
            onesh = mp.tile([P, P], FP16, tag="onesh")
            nc.sync.dma_start(out=onesh[:], in_=onesh_d[:])
            eps_t = mp.tile([P, 1], F32, tag="eps")
            nc.vector.memset(eps_t, EPS)
            # logit shift: exp(s-25) keeps softmax denominators within the
            # ACT Ln table's domain; numerator and denominator scale by the
            # same e^-25, so the softmax is unchanged
            sh_t = mp.tile([P, 1], F32, tag="sh")
            nc.vector.memset(sh_t, -25.0)
            ones1 = mp.tile([1, P], BF16, tag="ones1")
            nc.sync.dma_start(out=ones1[:], in_=ones1_d[:])
            bq_t = mp.tile([P, NE], F32, tag="bq")
            nc.sync.dma_start(out=bq_t[:], in_=bq_d[:])
            bp_t = mp.tile([P, NC], F32, tag="bp")
            nc.sync.dma_start(out=bp_t[:], in_=bp_d[:])
            b1_t = mp.tile([P, NN], F32, tag="b1")
            nc.sync.dma_start(out=b1_t[:], in_=b1_d[:])
            b2_t = mp.tile([P, NC], F32, tag="b2")
            nc.sync.dma_start(out=b2_t[:], in_=b2_d[:])

            xtf = mp.tile([P, NC, S], FP16, tag="xtf")  # x^T -> xn^T in place
            # 16 piece-DMAs in LN-job order: job j needs only pieces
            # (c, j) so the first stats matmuls start ~2us in
            for t0 in range(0, S, 512):
                for c in range(NC):
                    nc.sync.dma_start(out=xtf[:, c, t0:t0 + 512],
                                      in_=xtf_d[:, c, t0:t0 + 512])
            xto = mp.tile([P, NC, TO], F32, tag="xto")  # raw x^T ours (residual)
            nc.sync.dma_start(out=xto[:], in_=xto_d[:])
            wk_t = mp.tile([P, NC, D], FP16, tag="wk")
            nc.sync.dma_start(out=wk_t[:], in_=wk_d[:])
            wv_t = mp.tile([P, NC, D], FP16, tag="wv")
            nc.sync.dma_start(out=wv_t[:], in_=wv_d[:])
            wq_t = mp.tile([P, NC, D], FP16, tag="wq")
            nc.sync.dma_start(out=wq_t[:], in_=wq_d[:])
            wp_t = mp.tile([P, NC, D], FP16, tag="wp")
            nc.sync.dma_start(out=wp_t[:], in_=wp_d[:])

            # ---- transposed layernorm ----
            # jobs: list of (src, dst, t0, idx). Stats per 512-token slice via
            # all-ones stationary matmul (mean/meansq arrive broadcast on all
            # 128 partitions). src/dst bf16 -> DVE 2x mode on the elementwise.
            def ln_f32r(jobs):
                for srct, dstt, t0, i in jobs:
                    ps = pbig.tile([P, 2, 512], F32, tag="pb", name=f"lnps{i}")
                    for c in range(NC):
                        sl = srct[:, c, t0:t0 + 512]
                        sq = db.tile([P, 512], FP16, tag="sq", name=f"lnsq{i}")
                        nc.vector.tensor_tensor(sq[:], sl, sl, OP.mult)
                        nc.tensor.matmul(ps[:, 0, :], onesh[:], sl,
                                         start=(c == 0), stop=(c == NC - 1))
                        nc.tensor.matmul(ps[:, 1, :], onesh[:], sq[:],
                                         start=(c == 0), stop=(c == NC - 1))
                    mb = db.tile([P, 512], FP16, tag="mb", name=f"mb{i}")
                    nc.vector.tensor_copy(out=mb[:], in_=ps[:, 0, :])
                    rb = db.tile([P, 512], FP16, tag="rb", name=f"rb{i}")
                    nc.vector.tensor_tensor(rb[:], mb[:], mb[:], OP.mult)
                    nc.vector.tensor_tensor(rb[:], ps[:, 1, :], rb[:], OP.subtract)
                    # rstd = exp(-0.5*ln(var+eps)), in place
                    nc.scalar.activation(rb[:], rb[:], AF.Ln, bias=eps_t[:])
                    nc.scalar.activation(rb[:], rb[:], AF.Exp, scale=-0.5)
                    for c in range(NC):
                        sl_in = srct[:, c, t0:t0 + 512]
                        sl_out = dstt[:, c, t0:t0 + 512]
                        nc.vector.tensor_tensor(sl_out, sl_in, mb[:], OP.subtract)
                        nc.vector.tensor_tensor(sl_out, sl_out, rb[:], OP.mult)

            # ---- phase 1: LN1 on full sequence (in-place) ----
            ln_f32r([(xtf, xtf, t0, t0 // 512) for t0 in range(0, S, 512)])

            # ---- phase 3: QKV projections ----
            # k^T [e_all, t_full]; no bias (cancels in softmax)
            kt = mp.tile([P, NE, S], FP16, tag="kt")
            for e in range(NE):
                pss = [pbig.tile([P, 2, 512], F32, tag="pb", name=f"pss{e}_{i}")
                       for i in range(2)]
                for c in range(NC):
                    for tau in range(4):
                        nc.tensor.matmul(pss[tau // 2][:, tau % 2, :],
                                         (wk_t[:, c, e * P:(e + 1) * P]),
                                         (xtf[:, c, tau * 512:(tau + 1) * 512]),
                                         start=(c == 0), stop=(c == NC - 1))
                for tau in range(4):
                    dst = kt[:, e, tau * 512:(tau + 1) * 512]
                    src = pss[tau // 2][:, tau % 2, :]
                    if tau % 2 == 0:
                        nc.vector.tensor_copy(out=dst, in_=src)
                    else:
                        nc.scalar.copy(out=dst, in_=src)

            # v [t_full, 8 heads x (64 + ones + pad)] -- natural layout,
            # ones column makes the o-matmul also produce the softmax denom;
            # stride 66 keeps each head 4B-aligned for 2x DVE copies
            vv = mp.tile([P, NT, H, HD + 2], BF16, tag="vv")
            nc.vector.memset(vv[:, :, :, HD:HD + 1], 1.0)
            nc.vector.memset(vv[:, :, :, HD + 1:HD + 2], 0.0)
            for t in range(NT):
                ps = po.tile([P, 512], F32, tag="ps", name=f"vps{t}")
                for c in range(NC):
                    nc.tensor.matmul(ps[:],
                                     (xtf[:, c, t * P:(t + 1) * P]),
                                     (wv_t[:, c, :]),
                                     start=(c == 0), stop=(c == NC - 1))
                nc.vector.tensor_copy(
                    out=vv[:, t, :, 0:HD],
                    in_=ps[:].rearrange("p (h e) -> p h e", e=HD))

            # q^T per head, zero-padded to the full 128-partition e-sub so
            # the scores matmul contracts K=128; Q projects from the
            # normalized own-block slice of xtf
            qpA = mp.tile([P, NE, TO], FP16, tag="qpA")
            qpB = mp.tile([P, NE, TO], FP16, tag="qpB")
            for e in range(NE):
                nc.sync.dma_start(out=qpA[HD:P, e, :], in_=zer_d[:])
                nc.sync.dma_start(out=qpB[0:HD, e, :], in_=zer_d[:])
            for e in range(NE):
                ps = po.tile([P, 512], F32, tag="ps", name=f"qps{e}")
                for c in range(NC):
                    nc.tensor.matmul(ps[:],
                                     (wq_t[:, c, e * P:(e + 1) * P]),
                                     (xtf[:, c, 0:TO]),
                                     start=(c == 0), stop=(c == NC - 1))
                nc.vector.tensor_scalar_add(qpA[0:HD, e, :], ps[0:HD, :],
                                            bq_t[0:HD, e:e + 1])
                nc.vector.tensor_scalar_add(qpB[HD:P, e, :], ps[HD:P, :],
                                            bq_t[HD:P, e:e + 1])

            # prefetch W1 into the slot vacated by xn^T (tag xtf)
            w1_t = mp.tile([P, NC, DFF], FP16, tag="xtf", name="w1_t")
            nc.sync.dma_start(out=w1_t[:], in_=w1_d[:])

            # ---- phase 4: attention (scores^T, exp, o accumulate) ----
            ot = mp.tile([P, NC, TO], FP16, tag="ot")  # o^T packed [he, s]
            for e in range(NE):
                opss = []
                for hh in range(2):
                    h = 2 * e + hh
                    ops = po.tile([P, 512], F32, tag="ps", name=f"po{h}")
                    opss.append(ops)
                    for g in range(8):
                        scp = pbig.tile([P, 2, 512], F32, tag="pb",
                                        name=f"scp{h}_{g}")
                        for j in range(2):
                            k = 2 * g + j
                            nc.tensor.matmul(scp[:, j, :],
                                             (kt[:, e, k * P:(k + 1) * P]),
                                             (qpA[:, e, :] if hh == 0
                                              else qpB[:, e, :]),
                                             start=True, stop=True)
                        ex = expp.tile([P, 2, 512], BF16, tag="ex",
                                       name=f"ex{h}_{g}")
                        nc.scalar.activation(ex[:], scp[:], AF.Exp, bias=sh_t[:])
                        for j in range(2):
                            k = 2 * g + j
                            nc.tensor.matmul(ops[0:HD + 1, :],
                                             (vv[:, k, h, 0:HD + 1]),
                                             (ex[:, j, :]),
                                             start=(k == 0), stop=(k == NT - 1))
                    # evacuate numerators + denominator row to SBUF right
                    # away; the PSUM o-accumulator frees so the next head's
                    # o-matmuls never stall on this head's normalization
                    onum = db.tile([HD, 512], F32, tag="onum",
                                   name=f"onum{h}")
                    nc.vector.tensor_copy(out=onum[:], in_=ops[0:HD, :])
                    dr = db.tile([1, 512], F32, tag="dr", name=f"dr{h}")
                    nc.vector.tensor_copy(out=dr[:], in_=ops[HD:HD + 1, :])
                    # 1/d via fast DVE reciprocal (18-bit accurate), keeping
                    # ScalarE free for the attention exps
                    nc.vector.reciprocal_approx_fast(out=dr[:], in_=dr[:])
                    drb = db.tile([1, 512], BF16, tag="drb", name=f"drb{h}")
                    nc.vector.tensor_copy(out=drb[:], in_=dr[:])
                    # broadcast 1/d to 64 partitions with a K=1 ones matmul
                    rps = po.tile([P, 512], F32, tag="ps", name=f"rps{h}")
                    nc.tensor.matmul(rps[0:HD, :], ones1[:, 0:HD], drb[:],
                                     start=True, stop=True)
                    rbc = db.tile([HD, 512], F32, tag="rbc", name=f"rbc{h}")
                    nc.vector.tensor_copy(out=rbc[:], in_=rps[0:HD, :])
                    nc.vector.tensor_tensor(
                        ot[hh * HD:(hh + 1) * HD, e, :],
                        onum[:], rbc[:], OP.mult)

            # prefetch W2 into the slot vacated by k^T (tag kt)
            w2_t = mp.tile([P, NN, D], FP16, tag="kt", name="w2_t")
            nc.sync.dma_start(out=w2_t[:], in_=w2_d[:])

            # ---- phase 5: output projection + residual (transposed) ----
            xat = mp.tile([P, NC, TO], F32, tag="xat")
            for dsub in range(NC):
                ps = po.tile([P, 512], F32, tag="ps", name=f"pjps{dsub}")
                for hc in range(NC):
                    nc.tensor.matmul(ps[:],
                                     (wp_t[:, hc, dsub * P:(dsub + 1) * P]),
                                     (ot[:, hc, :]),
                                     start=(hc == 0), stop=(hc == NC - 1))
                nc.vector.scalar_tensor_tensor(
                    xat[:, dsub, :], ps[:], bp_t[:, dsub:dsub + 1],
                    xto[:, dsub, :], OP.add, OP.add)

            # ---- phase 6: LN2 (stats on a bf16 shadow of the residual) ----
            xatb = mp.tile([P, NC, TO], FP16, tag="xatb")
            for c in range(NC):
                nc.scalar.copy(out=xatb[:, c, :], in_=xat[:, c, :])
            xn2 = mp.tile([P, NC, TO], FP16, tag="xn2")
            ps2 = pbig.tile([P, 2, 512], F32, tag="pb", name="ln2ps")
            for c in range(NC):
                sq3 = db.tile([P, 512], FP16, tag="sq3", name="ln2sq")
                nc.vector.tensor_tensor(sq3[:], xatb[:, c, :], xatb[:, c, :],
                                        OP.mult)
                nc.tensor.matmul(ps2[:, 0, :], onesh[:], xatb[:, c, :],
                                 start=(c == 0), stop=(c == NC - 1))
                nc.tensor.matmul(ps2[:, 1, :], onesh[:], sq3[:],
                                 start=(c == 0), stop=(c == NC - 1))
            mb2 = mp.tile([P, 512], FP16, tag="lnmb2")
            nc.vector.tensor_copy(out=mb2[:], in_=ps2[:, 0, :])
            rb2 = mp.tile([P, 512], FP16, tag="lnrb2")
            nc.vector.tensor_tensor(rb2[:], mb2[:], mb2[:], OP.mult)
            nc.vector.tensor_tensor(rb2[:], ps2[:, 1, :], rb2[:], OP.subtract)
            nc.scalar.activation(rb2[:], rb2[:], AF.Ln, bias=eps_t[:])
            nc.scalar.activation(rb2[:], rb2[:], AF.Exp, scale=-0.5)
            for c in range(NC):
                nc.vector.tensor_tensor(xn2[:, c, :], xatb[:, c, :], mb2[:],
                                        OP.subtract)
                nc.vector.tensor_tensor(xn2[:, c, :], xn2[:, c, :], rb2[:],
                                        OP.mult)

            # ---- phase 7: MLP in -- h^T = relu(W1^T xn2 + b1) on ScalarE ----
            ht = mp.tile([P, NN, TO], FP16, tag="ht")
            for n in range(NN):
                ps = po.tile([P, 512], F32, tag="ps", name=f"h1ps{n}")
                for c in range(NC):
                    nc.tensor.matmul(ps[:],
                                     (w1_t[:, c, n * P:(n + 1) * P]),
                                     (xn2[:, c, :]),
                                     start=(c == 0), stop=(c == NC - 1))
                nc.scalar.activation(ht[:, n, :], ps[:], AF.Relu,
                                     bias=b1_t[:, n:n + 1])

            # ---- phase 8: MLP out + residual, y^T [d, t] ----
            # yt reuses xto's slot (residual dead after the proj add)
            yt = mp.tile([P, NC, TO], F32, tag="xto", name="yt")
            for dsub in range(NC):
                ps = po.tile([P, 512], F32, tag="ps", name=f"h2ps{dsub}")
                for n in range(NN):
                    nc.tensor.matmul(ps[:],
                                     (w2_t[:, n, dsub * P:(dsub + 1) * P]),
                                     (ht[:, n, :]),
                                     start=(n == 0), stop=(n == NN - 1))
                nc.vector.scalar_tensor_tensor(
                    yt[:, dsub, :], ps[:], b2_t[:, dsub:dsub + 1],
                    xat[:, dsub, :], OP.add, OP.add)
                nc.sync.dma_start(out=yt_d[:, dsub, :], in_=yt[:, dsub, :])

    return nc


def _chunk_p(a):
    """[K, N] -> [128, K//128, N] (partition-major SBUF layout)."""
    K = a.shape[0]
    return np.ascontiguousarray(
        a.reshape(K // P, P, *a.shape[1:]).transpose(1, 0, *range(2, a.ndim + 1)))


def host_inputs(x, Wq, bq, Wk, bk, Wv, bv, Wp, bp, W1, b1, W2, b2,
                g1, be1, g2, be2):
    """Fold LN affines into the projections, pre-transpose/chunk everything.

    bk is dropped (constant-per-query score shift cancels in softmax).
    bv is folded into bp: o = o_nodiv/denom + bv, so attn = o@Wp + bp
    becomes o_div@Wp + (bp + bv@Wp).
    """
    f = np.float32
    Wq_all = np.ascontiguousarray(Wq.transpose(1, 0, 2).reshape(D, D), f)
    Wk_all = np.ascontiguousarray(Wk.transpose(1, 0, 2).reshape(D, D), f)
    Wv_all = np.ascontiguousarray(Wv.transpose(1, 0, 2).reshape(D, D), f)
    bq_all = (bq.reshape(D) + be1 @ Wq_all).astype(f)
    bv_all = (bv.reshape(D) + be1 @ Wv_all).astype(f)
    Wq_f = Wq_all * g1[:, None]
    Wk_f = Wk_all * g1[:, None]
    Wv_f = Wv_all * g1[:, None]
    W1_f = (W1 * g2[:, None]).astype(f)
    b1_f = (b1 + be2 @ W1).astype(f)
    bp_f = (bp + bv_all @ Wp).astype(f)

    shared = {
        "wq": _chunk_p(Wq_f.astype(f)).astype(F16),
        "wk": _chunk_p(Wk_f.astype(f)).astype(F16),
        "wv": _chunk_p(Wv_f.astype(f)).astype(F16),
        "wp": _chunk_p(Wp.astype(f)).astype(F16),
        "w1": _chunk_p(W1_f).astype(F16),
        "w2": _chunk_p(W2.astype(f)).astype(F16),
        "bq": np.ascontiguousarray(bq_all.reshape(NE, P).T),
        "bp": np.ascontiguousarray(bp_f.reshape(NC, P).T),
        "b1": np.ascontiguousarray(b1_f.reshape(NN, P).T),
        "b2": np.ascontiguousarray(b2.astype(f).reshape(NC, P).T),
        "onesh": np.full((P, P), 1.0 / D, F16),
        "zer64": np.zeros((HD, TO), F16),
        "ones1": np.ones((1, P), BF),
    }
    in_maps = []
    for c in range(8):
        b, qb = c // 4, c % 4
        # roll so each core's own query block sits first: Q projection
        # always reads xtf[:, :, 0:TO]
        xT = np.ascontiguousarray(
            np.roll(x[b].T.astype(f), -qb * TO, axis=1))    # [D, S]
        m = dict(shared)
        m["xtf"] = _chunk_p(xT).astype(F16)
        m["xto"] = _chunk_p(np.ascontiguousarray(xT[:, 0:TO]))
        in_maps.append(m)
    return in_maps


def assemble_output(results, dtype):
    y = np.empty((B, S, D), np.float32)
    for c in range(8):
        b, qb = c // 4, c % 4
        yt = results[c]["yt"]                                 # [P, NC, TO]
        yT = yt.transpose(1, 0, 2).reshape(D, TO)             # [D, TO]
        y[b, qb * TO:(qb + 1) * TO, :] = yT.T
    return y.astype(dtype, copy=False)


def _enable_ntff_trace():
    """The image's `antenv` lacks `axon_hooks`; inject it and register the
    ctypes NTFF profile hook from trn_boot so trace=True yields exec times."""
    import types
    if "antenv.axon_hooks" not in sys.modules:
        mod = types.ModuleType("antenv.axon_hooks")
        mod._hook = None
        mod.set_axon_ntff_profile_hook = lambda h: setattr(mod, "_hook", h)
        mod.get_axon_ntff_profile_hook = lambda: mod._hook
        sys.modules["antenv.axon_hooks"] = mod
    import antenv.axon_hooks as ah
    if ah.get_axon_ntff_profile_hook() is None:
        try:
            from trn_agent_boot.trn_boot import _ntff_profile_via_ctypes
            ah.set_axon_ntff_profile_hook(
                _ntff_profile_via_ctypes("/opt/axon/libaxon_pjrt.so"))
        except Exception:
            pass
    import concourse.bass_utils as bu
    bu.upload_artifacts = lambda d: d  # no artifact bucket in this container


def kernel(**inputs) -> np.ndarray:
    global LAST_EXEC_NS
    in_maps = host_inputs(**{k: np.asarray(v) for k, v in inputs.items()})
    nc = build_program()
    nc.finalize()
    trace = os.environ.get("KERNEL_TRACE", "0") == "1"
    kwargs = {}
    if trace:
        _enable_ntff_trace()
        tmpdir = os.environ.get("KERNEL_TRACE_DIR", "/tmp/ktrace")
        os.makedirs(tmpdir, exist_ok=True)
        kwargs["tmpdir"] = tmpdir
    res = run_bass_kernel_spmd(nc, in_maps, list(range(8)), trace=trace, **kwargs)
    LAST_EXEC_NS = res.exec_time_ns
    return assemble_output(res.results, np.asarray(inputs["x"]).dtype)


# revision 30
# speedup vs baseline: 1.1920x; 1.1920x over previous
"""Fused transformer block (pre-norm attention + MLP) for Trainium2, 8 NeuronCores.

Problem: B=2, S=2048, D=512, H=8, HD=64, fp32 in/out.

Sharding: pure data-parallel over (batch, query-block). Core c handles batch
b = c//4 and query rows [qb*512, (qb+1)*512) with qb = c%4. Each core
redundantly computes LN1 + K/V for the full 2048-token sequence of its batch
element (no collectives needed).

v2 changes vs baseline:
- bf16 datapath for every matmul (weights pre-cast on host; activations
  written bf16 at PSUM evacuation). PSUM accumulation stays fp32. Residual
  stream (xto/xat/yt) stays fp32.
- K bias dropped entirely (constant-per-query score shift cancels in
  softmax); V bias folded into the proj bias on host (bp' = bp + bv@Wp).
- The redundant own-block re-normalization (xno) removed; Q projects from
  the already-normalized xtf slice.
- MLP ReLU+bias moved to ScalarE (Relu is in the natural_log_exp table set,
  so no table reload); frees the Vector engine.
- LN math in bf16 2x DVE mode where operands live in SBUF.
- LN2 stats use f32r bitcast views of the fp32 residual (no cast pass).
"""

import os
import sys

sys.path.insert(0, "/opt/trn_rl_repo")

import numpy as np
import ml_dtypes

import concourse.bass as bass
import concourse.mybir as mybir
import concourse.tile as tile
from concourse import bacc
from concourse.bass_utils import run_bass_kernel_spmd

P = 128
B, S, D, H = 2, 2048, 512, 8
HD = D // H          # 64
DFF = 4 * D          # 2048
TO = 512             # tokens owned per core (query block)
NC = D // P          # 4 d-chunks
NE = D // P          # 4 e_all-subs (q/k head pairs)
NT = S // P          # 16 t-subs
NN = DFF // P        # 16 n-subs
EPS = 1e-5
F32 = mybir.dt.float32
F32R = mybir.dt.float32r
BF16 = mybir.dt.bfloat16
AF = mybir.ActivationFunctionType
OP = mybir.AluOpType
BF = ml_dtypes.bfloat16
FP16 = mybir.dt.float16
F16 = np.float16

LAST_EXEC_NS = None

# Force every Exp/Ln ACTIVATE onto the one table set containing both, so the
# ACT table never reloads mid-kernel (each reload is ~2.7us and serializes
# against the attention exps). Copy/Identity/Relu/Square are in every set.
_orig_gat = bacc.get_activation_tables


def _gat_nlexp_only(arch):
    tabs = _orig_gat(arch)
    for name, fns in tabs.items():
        if name != "natural_log_exp_and_others":
            fns.discard(AF.Exp)
            fns.discard(AF.Ln)
    return tabs


bacc.get_activation_tables = _gat_nlexp_only

def build_program():
    nc = bacc.Bacc("TRN2", target_bir_lowering=False, debug=False, num_devices=8)

    xtf_d = nc.dram_tensor("xtf", [P, NC, S], FP16, kind="ExternalInput")
    xto_d = nc.dram_tensor("xto", [P, NC, TO], F32, kind="ExternalInput")
    wq_d = nc.dram_tensor("wq", [P, NC, D], FP16, kind="ExternalInput")
    wk_d = nc.dram_tensor("wk", [P, NC, D], FP16, kind="ExternalInput")
    wv_d = nc.dram_tensor("wv", [P, NC, D], FP16, kind="ExternalInput")
    wp_d = nc.dram_tensor("wp", [P, NC, D], FP16, kind="ExternalInput")
    w1_d = nc.dram_tensor("w1", [P, NC, DFF], FP16, kind="ExternalInput")
    w2_d = nc.dram_tensor("w2", [P, NN, D], FP16, kind="ExternalInput")
    bq_d = nc.dram_tensor("bq", [P, NE], F32, kind="ExternalInput")
    bp_d = nc.dram_tensor("bp", [P, NC], F32, kind="ExternalInput")
    b1_d = nc.dram_tensor("b1", [P, NN], F32, kind="ExternalInput")
    b2_d = nc.dram_tensor("b2", [P, NC], F32, kind="ExternalInput")
    onesh_d = nc.dram_tensor("onesh", [P, P], FP16, kind="ExternalInput")
    zer_d = nc.dram_tensor("zer64", [HD, TO], FP16, kind="ExternalInput")
    ones1_d = nc.dram_tensor("ones1", [1, P], BF16, kind="ExternalInput")
    yt_d = nc.dram_tensor("yt", [P, NC, TO], F32, kind="ExternalOutput")

    with tile.TileContext(nc) as tc:
        with (
            tc.tile_pool(name="main", bufs=1) as mp,
            tc.tile_pool(name="db", bufs=2) as db,
            tc.tile_pool(name="expp", bufs=4) as expp,
            tc.tile_pool(name="pbig", bufs=3, space="PSUM") as pbig,
            tc.tile_pool(name="po", bufs=2, space="PSUM") as po,
        ):
            # ---- constants / weights (all prefetched up front) ----
            onesh = mp.tile([P, P], FP16, tag="onesh")
            nc.sync.dma_start(out=onesh[:], in_=onesh_d[:])
            eps_t = mp.tile([P, 1], F32, tag="eps")
            nc.vector.memset(eps_t, EPS)
            # logit shift: exp(s-25) keeps softmax denominators within the
            # ACT Ln table's domain; numerator and denominator scale by the
            # same e^-25, so the softmax is unchanged
            sh_t = mp.tile([P, 1], F32, tag="sh")
            nc.vector.memset(sh_t, -25.0)
            ones1 = mp.tile([1, P], BF16, tag="ones1")
            nc.sync.dma_start(out=ones1[:], in_=ones1_d[:])
            bq_t = mp.tile([P, NE], F32, tag="bq")
            nc.sync.dma_start(out=bq_t[:], in_=bq_d[:])
            bp_t = mp.tile([P, NC], F32, tag="bp")
            nc.sync.dma_start(out=bp_t[:], in_=bp_d[:])
            b1_t = mp.tile([P, NN], F32, tag="b1")
            nc.sync.dma_start(out=b1_t[:], in_=b1_d[:])
            b2_t = mp.tile([P, NC], F32, tag="b2")
            nc.sync.dma_start(out=b2_t[:], in_=b2_d[:])

            xtf = mp.tile([P, NC, S], FP16, tag="xtf")  # x^T -> xn^T in place
            # 16 piece-DMAs in LN-job order: job j needs only pieces
            # (c, j) so the first stats matmuls start ~2us in
            for t0 in range(0, S, 512):
                for c in range(NC):
                    nc.sync.dma_start(out=xtf[:, c, t0:t0 + 512],
                                      in_=xtf_d[:, c, t0:t0 + 512])
            xto = mp.tile([P, NC, TO], F32, tag="xto")  # raw x^T ours (residual)
            nc.sync.dma_start(out=xto[:], in_=xto_d[:])
            wk_t = mp.tile([P, NC, D], FP16, tag="wk")
            nc.sync.dma_start(out=wk_t[:], in_=wk_d[:])
            wv_t = mp.tile([P, NC, D], FP16, tag="wv")
            nc.sync.dma_start(out=wv_t[:], in_=wv_d[:])
            wq_t = mp.tile([P, NC, D], FP16, tag="wq")
            nc.sync.dma_start(out=wq_t[:], in_=wq_d[:])
            wp_t = mp.tile([P, NC, D], FP16, tag="wp")
            nc.sync.dma_start(out=wp_t[:], in_=wp_d[:])

            # ---- transposed layernorm ----
            # jobs: list of (src, dst, t0, idx). Stats per 512-token slice via
            # all-ones stationary matmul (mean/meansq arrive broadcast on all
            # 128 partitions). src/dst bf16 -> DVE 2x mode on the elementwise.
            def ln_f32r(jobs):
                for srct, dstt, t0, i in jobs:
                    ps = pbig.tile([P, 2, 512], F32, tag="pb", name=f"lnps{i}")
                    for c in range(NC):
                        sl = srct[:, c, t0:t0 + 512]
                        sq = db.tile([P, 512], FP16, tag="sq", name=f"lnsq{i}")
                        nc.scalar.square(sq[:], sl)
                        nc.tensor.matmul(ps[:, 0, :], onesh[:], sl,
                                         start=(c == 0), stop=(c == NC - 1))
                        nc.tensor.matmul(ps[:, 1, :], onesh[:], sq[:],
                                         start=(c == 0), stop=(c == NC - 1))
                    mb = db.tile([P, 512], FP16, tag="mb", name=f"mb{i}")
                    nc.scalar.copy(out=mb[:], in_=ps[:, 0, :])
                    rb = db.tile([P, 512], FP16, tag="rb", name=f"rb{i}")
                    nc.vector.tensor_tensor(rb[:], mb[:], mb[:], OP.mult)
                    nc.vector.tensor_tensor(rb[:], ps[:, 1, :], rb[:], OP.subtract)
                    # rstd = exp(-0.5*ln(var+eps)), in place
                    nc.scalar.activation(rb[:], rb[:], AF.Ln, bias=eps_t[:])
                    nc.scalar.activation(rb[:], rb[:], AF.Exp, scale=-0.5)
                    for c in range(NC):
                        sl_in = srct[:, c, t0:t0 + 512]
                        sl_out = dstt[:, c, t0:t0 + 512]
                        nc.vector.tensor_tensor(sl_out, sl_in, mb[:], OP.subtract)
                        nc.vector.tensor_tensor(sl_out, sl_out, rb[:], OP.mult)

            # ---- phase 1: LN1 on full sequence (in-place) ----
            ln_f32r([(xtf, xtf, t0, t0 // 512) for t0 in range(0, S, 512)])

            # ---- phase 3: QKV projections ----
            # k^T [e_all, t_full]; no bias (cancels in softmax)
            kt = mp.tile([P, NE, S], FP16, tag="kt")
            for e in range(NE):
                pss = [pbig.tile([P, 2, 512], F32, tag="pb", name=f"pss{e}_{i}")
                       for i in range(2)]
                for c in range(NC):
                    for tau in range(4):
                        nc.tensor.matmul(pss[tau // 2][:, tau % 2, :],
                                         (wk_t[:, c, e * P:(e + 1) * P]),
                                         (xtf[:, c, tau * 512:(tau + 1) * 512]),
                                         start=(c == 0), stop=(c == NC - 1))
                for tau in range(4):
                    dst = kt[:, e, tau * 512:(tau + 1) * 512]
                    src = pss[tau // 2][:, tau % 2, :]
                    if tau % 2 == 0:
                        nc.vector.tensor_copy(out=dst, in_=src)
                    else:
                        nc.scalar.copy(out=dst, in_=src)

            # v [t_full, 8 heads x (64 + ones + pad)] -- natural layout,
            # ones column makes the o-matmul also produce the softmax denom;
            # stride 66 keeps each head 4B-aligned for 2x DVE copies
            vv = mp.tile([P, NT, H, HD + 2], BF16, tag="vv")
            nc.vector.memset(vv[:, :, :, HD:HD + 1], 1.0)
            nc.vector.memset(vv[:, :, :, HD + 1:HD + 2], 0.0)
            for t in range(NT):
                ps = po.tile([P, 512], F32, tag="ps", name=f"vps{t}")
                for c in range(NC):
                    nc.tensor.matmul(ps[:],
                                     (xtf[:, c, t * P:(t + 1) * P]),
                                     (wv_t[:, c, :]),
                                     start=(c == 0), stop=(c == NC - 1))
                nc.vector.tensor_copy(
                    out=vv[:, t, :, 0:HD],
                    in_=ps[:].rearrange("p (h e) -> p h e", e=HD))

            # q^T per head, zero-padded to the full 128-partition e-sub so
            # the scores matmul contracts K=128; Q projects from the
            # normalized own-block slice of xtf
            qpA = mp.tile([P, NE, TO], FP16, tag="qpA")
            qpB = mp.tile([P, NE, TO], FP16, tag="qpB")
            for e in range(NE):
                nc.sync.dma_start(out=qpA[HD:P, e, :], in_=zer_d[:])
                nc.sync.dma_start(out=qpB[0:HD, e, :], in_=zer_d[:])
            for e in range(NE):
                ps = po.tile([P, 512], F32, tag="ps", name=f"qps{e}")
                for c in range(NC):
                    nc.tensor.matmul(ps[:],
                                     (wq_t[:, c, e * P:(e + 1) * P]),
                                     (xtf[:, c, 0:TO]),
                                     start=(c == 0), stop=(c == NC - 1))
                nc.vector.tensor_scalar_add(qpA[0:HD, e, :], ps[0:HD, :],
                                            bq_t[0:HD, e:e + 1])
                nc.vector.tensor_scalar_add(qpB[HD:P, e, :], ps[HD:P, :],
                                            bq_t[HD:P, e:e + 1])

            # prefetch W1 into the slot vacated by xn^T (tag xtf)
            w1_t = mp.tile([P, NC, DFF], FP16, tag="xtf", name="w1_t")
            nc.sync.dma_start(out=w1_t[:], in_=w1_d[:])

            # ---- phase 4: attention (scores^T, exp, o accumulate) ----
            ot = mp.tile([P, NC, TO], FP16, tag="ot")  # o^T packed [he, s]
            for e in range(NE):
                opss = []
                for hh in range(2):
                    h = 2 * e + hh
                    ops = po.tile([P, 512], F32, tag="ps", name=f"po{h}")
                    opss.append(ops)
                    for g in range(8):
                        scp = pbig.tile([P, 2, 512], F32, tag="pb",
                                        name=f"scp{h}_{g}")
                        for j in range(2):
                            k = 2 * g + j
                            nc.tensor.matmul(scp[:, j, :],
                                             (kt[:, e, k * P:(k + 1) * P]),
                                             (qpA[:, e, :] if hh == 0
                                              else qpB[:, e, :]),
                                             start=True, stop=True)
                        ex = expp.tile([P, 2, 512], BF16, tag="ex",
                                       name=f"ex{h}_{g}")
                        nc.scalar.activation(ex[:], scp[:], AF.Exp, bias=sh_t[:])
                        for j in range(2):
                            k = 2 * g + j
                            nc.tensor.matmul(ops[0:HD + 1, :],
                                             (vv[:, k, h, 0:HD + 1]),
                                             (ex[:, j, :]),
                                             start=(k == 0), stop=(k == NT - 1))
                    # evacuate numerators + denominator row to SBUF right
                    # away; the PSUM o-accumulator frees so the next head's
                    # o-matmuls never stall on this head's normalization
                    onum = db.tile([HD, 512], F32, tag="onum",
                                   name=f"onum{h}")
                    nc.vector.tensor_copy(out=onum[:], in_=ops[0:HD, :])
                    dr = db.tile([1, 512], F32, tag="dr", name=f"dr{h}")
                    nc.vector.tensor_copy(out=dr[:], in_=ops[HD:HD + 1, :])
                    # 1/d via fast DVE reciprocal (18-bit accurate), keeping
                    # ScalarE free for the attention exps
                    nc.vector.reciprocal_approx_fast(out=dr[:], in_=dr[:])
                    drb = db.tile([1, 512], BF16, tag="drb", name=f"drb{h}")
                    nc.vector.tensor_copy(out=drb[:], in_=dr[:])
                    # broadcast 1/d to 64 partitions with a K=1 ones matmul
                    rps = po.tile([P, 512], F32, tag="ps", name=f"rps{h}")
                    nc.tensor.matmul(rps[0:HD, :], ones1[:, 0:HD], drb[:],
                                     start=True, stop=True)
                    rbc = db.tile([HD, 512], F32, tag="rbc", name=f"rbc{h}")
                    nc.vector.tensor_copy(out=rbc[:], in_=rps[0:HD, :])
                    nc.vector.tensor_tensor(
                        ot[hh * HD:(hh + 1) * HD, e, :],
                        onum[:], rbc[:], OP.mult)

            # prefetch W2 into the slot vacated by k^T (tag kt)
            w2_t = mp.tile([P, NN, D], FP16, tag="kt", name="w2_t")
            nc.sync.dma_start(out=w2_t[:], in_=w2_d[:])

            # ---- phase 5: output projection + residual (transposed) ----
            xat = mp.tile([P, NC, TO], F32, tag="xat")
            for dsub in range(NC):
                ps = po.tile([P, 512], F32, tag="ps", name=f"pjps{dsub}")
                for hc in range(NC):
                    nc.tensor.matmul(ps[:],
                                     (wp_t[:, hc, dsub * P:(dsub + 1) * P]),
                                     (ot[:, hc, :]),
                                     start=(hc == 0), stop=(hc == NC - 1))
                nc.vector.scalar_tensor_tensor(
                    xat[:, dsub, :], ps[:], bp_t[:, dsub:dsub + 1],
                    xto[:, dsub, :], OP.add, OP.add)

            # ---- phase 6: LN2 (stats on a bf16 shadow of the residual) ----
            xatb = mp.tile([P, NC, TO], FP16, tag="xatb")
            for c in range(NC):
                nc.scalar.copy(out=xatb[:, c, :], in_=xat[:, c, :])
            xn2 = mp.tile([P, NC, TO], FP16, tag="xn2")
            ps2 = pbig.tile([P, 2, 512], F32, tag="pb", name="ln2ps")
            for c in range(NC):
                sq3 = db.tile([P, 512], FP16, tag="sq3", name="ln2sq")
                nc.scalar.square(sq3[:], xatb[:, c, :])
                nc.tensor.matmul(ps2[:, 0, :], onesh[:], xatb[:, c, :],
                                 start=(c == 0), stop=(c == NC - 1))
                nc.tensor.matmul(ps2[:, 1, :], onesh[:], sq3[:],
                                 start=(c == 0), stop=(c == NC - 1))
            mb2 = mp.tile([P, 512], FP16, tag="lnmb2")
            nc.scalar.copy(out=mb2[:], in_=ps2[:, 0, :])
            rb2 = mp.tile([P, 512], FP16, tag="lnrb2")
            nc.vector.tensor_tensor(rb2[:], mb2[:], mb2[:], OP.mult)
            nc.vector.tensor_tensor(rb2[:], ps2[:, 1, :], rb2[:], OP.subtract)
            nc.scalar.activation(rb2[:], rb2[:], AF.Ln, bias=eps_t[:])
            nc.scalar.activation(rb2[:], rb2[:], AF.Exp, scale=-0.5)
            for c in range(NC):
                nc.vector.tensor_tensor(xn2[:, c, :], xatb[:, c, :], mb2[:],
                                        OP.subtract)
                nc.vector.tensor_tensor(xn2[:, c, :], xn2[:, c, :], rb2[:],
                                        OP.mult)

            # ---- phase 7: MLP in -- h^T = relu(W1^T xn2 + b1) on ScalarE ----
            ht = mp.tile([P, NN, TO], FP16, tag="ht")
            for n in range(NN):
                ps = po.tile([P, 512], F32, tag="ps", name=f"h1ps{n}")
                for c in range(NC):
                    nc.tensor.matmul(ps[:],
                                     (w1_t[:, c, n * P:(n + 1) * P]),
                                     (xn2[:, c, :]),
                                     start=(c == 0), stop=(c == NC - 1))
                nc.scalar.activation(ht[:, n, :], ps[:], AF.Relu,
                                     bias=b1_t[:, n:n + 1])

            # ---- phase 8: MLP out + residual, y^T [d, t] ----
            # yt reuses xto's slot (residual dead after the proj add)
            yt = mp.tile([P, NC, TO], F32, tag="xto", name="yt")
            for dsub in range(NC):
                ps = po.tile([P, 512], F32, tag="ps", name=f"h2ps{dsub}")
                for n in range(NN):
                    nc.tensor.matmul(ps[:],
                                     (w2_t[:, n, dsub * P:(dsub + 1) * P]),
                                     (ht[:, n, :]),
                                     start=(n == 0), stop=(n == NN - 1))
                nc.vector.scalar_tensor_tensor(
                    yt[:, dsub, :], ps[:], b2_t[:, dsub:dsub + 1],
                    xat[:, dsub, :], OP.add, OP.add)
                nc.sync.dma_start(out=yt_d[:, dsub, :], in_=yt[:, dsub, :])

    return nc


def _chunk_p(a):
    """[K, N] -> [128, K//128, N] (partition-major SBUF layout)."""
    K = a.shape[0]
    return np.ascontiguousarray(
        a.reshape(K // P, P, *a.shape[1:]).transpose(1, 0, *range(2, a.ndim + 1)))


def host_inputs(x, Wq, bq, Wk, bk, Wv, bv, Wp, bp, W1, b1, W2, b2,
                g1, be1, g2, be2):
    """Fold LN affines into the projections, pre-transpose/chunk everything.

    bk is dropped (constant-per-query score shift cancels in softmax).
    bv is folded into bp: o = o_nodiv/denom + bv, so attn = o@Wp + bp
    becomes o_div@Wp + (bp + bv@Wp).
    """
    f = np.float32
    Wq_all = np.ascontiguousarray(Wq.transpose(1, 0, 2).reshape(D, D), f)
    Wk_all = np.ascontiguousarray(Wk.transpose(1, 0, 2).reshape(D, D), f)
    Wv_all = np.ascontiguousarray(Wv.transpose(1, 0, 2).reshape(D, D), f)
    bq_all = (bq.reshape(D) + be1 @ Wq_all).astype(f)
    bv_all = (bv.reshape(D) + be1 @ Wv_all).astype(f)
    Wq_f = Wq_all * g1[:, None]
    Wk_f = Wk_all * g1[:, None]
    Wv_f = Wv_all * g1[:, None]
    W1_f = (W1 * g2[:, None]).astype(f)
    b1_f = (b1 + be2 @ W1).astype(f)
    bp_f = (bp + bv_all @ Wp).astype(f)

    shared = {
        "wq": _chunk_p(Wq_f.astype(f)).astype(F16),
        "wk": _chunk_p(Wk_f.astype(f)).astype(F16),
        "wv": _chunk_p(Wv_f.astype(f)).astype(F16),
        "wp": _chunk_p(Wp.astype(f)).astype(F16),
        "w1": _chunk_p(W1_f).astype(F16),
        "w2": _chunk_p(W2.astype(f)).astype(F16),
        "bq": np.ascontiguousarray(bq_all.reshape(NE, P).T),
        "bp": np.ascontiguousarray(bp_f.reshape(NC, P).T),
        "b1": np.ascontiguousarray(b1_f.reshape(NN, P).T),
        "b2": np.ascontiguousarray(b2.astype(f).reshape(NC, P).T),
        "onesh": np.full((P, P), 1.0 / D, F16),
        "zer64": np.zeros((HD, TO), F16),
        "ones1": np.ones((1, P), BF),
    }
    in_maps = []
    for c in range(8):
        b, qb = c // 4, c % 4
        # roll so each core's own query block sits first: Q projection
        # always reads xtf[:, :, 0:TO]
        xT = np.ascontiguousarray(
            np.roll(x[b].T.astype(f), -qb * TO, axis=1))    # [D, S]
        m = dict(shared)
        m["xtf"] = _chunk_p(xT).astype(F16)
        m["xto"] = _chunk_p(np.ascontiguousarray(xT[:, 0:TO]))
        in_maps.append(m)
    return in_maps


def assemble_output(results, dtype):
    y = np.empty((B, S, D), np.float32)
    for c in range(8):
        b, qb = c // 4, c % 4
        yt = results[c]["yt"]                                 # [P, NC, TO]
        yT = yt.transpose(1, 0, 2).reshape(D, TO)             # [D, TO]
        y[b, qb * TO:(qb + 1) * TO, :] = yT.T
    return y.astype(dtype, copy=False)


def _enable_ntff_trace():
    """The image's `antenv` lacks `axon_hooks`; inject it and register the
    ctypes NTFF profile hook from trn_boot so trace=True yields exec times."""
    import types
    if "antenv.axon_hooks" not in sys.modules:
        mod = types.ModuleType("antenv.axon_hooks")
        mod._hook = None
        mod.set_axon_ntff_profile_hook = lambda h: setattr(mod, "_hook", h)
        mod.get_axon_ntff_profile_hook = lambda: mod._hook
        sys.modules["antenv.axon_hooks"] = mod
    import antenv.axon_hooks as ah
    if ah.get_axon_ntff_profile_hook() is None:
        try:
            from trn_agent_boot.trn_boot import _ntff_profile_via_ctypes
            ah.set_axon_ntff_profile_hook(
                _ntff_profile_via_ctypes("/opt/axon/libaxon_pjrt.so"))
        except Exception:
            pass
    import concourse.bass_utils as bu
    bu.upload_artifacts = lambda d: d  # no artifact bucket in this container


def kernel(**inputs) -> np.ndarray:
    global LAST_EXEC_NS
    in_maps = host_inputs(**{k: np.asarray(v) for k, v in inputs.items()})
    nc = build_program()
    nc.finalize()
    trace = os.environ.get("KERNEL_TRACE", "0") == "1"
    kwargs = {}
    if trace:
        _enable_ntff_trace()
        tmpdir = os.environ.get("KERNEL_TRACE_DIR", "/tmp/ktrace")
        os.makedirs(tmpdir, exist_ok=True)
        kwargs["tmpdir"] = tmpdir
    res = run_bass_kernel_spmd(nc, in_maps, list(range(8)), trace=trace, **kwargs)
    LAST_EXEC_NS = res.exec_time_ns
    return assemble_output(res.results, np.asarray(inputs["x"]).dtype)


# revision 31
# speedup vs baseline: 1.2192x; 1.0228x over previous
"""Fused transformer block (pre-norm attention + MLP) for Trainium2, 8 NeuronCores.

Problem: B=2, S=2048, D=512, H=8, HD=64, fp32 in/out.

Sharding: pure data-parallel over (batch, query-block). Core c handles batch
b = c//4 and query rows [qb*512, (qb+1)*512) with qb = c%4. Each core
redundantly computes LN1 + K/V for the full 2048-token sequence of its batch
element (no collectives needed).

v2 changes vs baseline:
- bf16 datapath for every matmul (weights pre-cast on host; activations
  written bf16 at PSUM evacuation). PSUM accumulation stays fp32. Residual
  stream (xto/xat/yt) stays fp32.
- K bias dropped entirely (constant-per-query score shift cancels in
  softmax); V bias folded into the proj bias on host (bp' = bp + bv@Wp).
- The redundant own-block re-normalization (xno) removed; Q projects from
  the already-normalized xtf slice.
- MLP ReLU+bias moved to ScalarE (Relu is in the natural_log_exp table set,
  so no table reload); frees the Vector engine.
- LN math in bf16 2x DVE mode where operands live in SBUF.
- LN2 stats use f32r bitcast views of the fp32 residual (no cast pass).
"""

import os
import sys

sys.path.insert(0, "/opt/trn_rl_repo")

import numpy as np
import ml_dtypes

import concourse.bass as bass
import concourse.mybir as mybir
import concourse.tile as tile
from concourse import bacc
from concourse.bass_utils import run_bass_kernel_spmd

P = 128
B, S, D, H = 2, 2048, 512, 8
HD = D // H          # 64
DFF = 4 * D          # 2048
TO = 512             # tokens owned per core (query block)
NC = D // P          # 4 d-chunks
NE = D // P          # 4 e_all-subs (q/k head pairs)
NT = S // P          # 16 t-subs
NN = DFF // P        # 16 n-subs
EPS = 1e-5
F32 = mybir.dt.float32
F32R = mybir.dt.float32r
BF16 = mybir.dt.bfloat16
AF = mybir.ActivationFunctionType
OP = mybir.AluOpType
BF = ml_dtypes.bfloat16
FP16 = mybir.dt.float16
F16 = np.float16

LAST_EXEC_NS = None

# Force every Exp/Ln ACTIVATE onto the one table set containing both, so the
# ACT table never reloads mid-kernel (each reload is ~2.7us and serializes
# against the attention exps). Copy/Identity/Relu/Square are in every set.
_orig_gat = bacc.get_activation_tables


def _gat_nlexp_only(arch):
    tabs = _orig_gat(arch)
    for name, fns in tabs.items():
        if name != "natural_log_exp_and_others":
            fns.discard(AF.Exp)
            fns.discard(AF.Ln)
    return tabs


bacc.get_activation_tables = _gat_nlexp_only

def build_program():
    nc = bacc.Bacc("TRN2", target_bir_lowering=False, debug=False, num_devices=8)

    xtf_d = nc.dram_tensor("xtf", [P, NC, S], FP16, kind="ExternalInput")
    xto_d = nc.dram_tensor("xto", [P, NC, TO], F32, kind="ExternalInput")
    wq_d = nc.dram_tensor("wq", [P, NC, D], FP16, kind="ExternalInput")
    wk_d = nc.dram_tensor("wk", [P, NC, D], FP16, kind="ExternalInput")
    wv_d = nc.dram_tensor("wv", [P, NC, D], FP16, kind="ExternalInput")
    wp_d = nc.dram_tensor("wp", [P, NC, D], FP16, kind="ExternalInput")
    w1_d = nc.dram_tensor("w1", [P, NC, DFF], FP16, kind="ExternalInput")
    w2_d = nc.dram_tensor("w2", [P, NN, D], FP16, kind="ExternalInput")
    bq_d = nc.dram_tensor("bq", [P, NE], F32, kind="ExternalInput")
    bp_d = nc.dram_tensor("bp", [P, NC], F32, kind="ExternalInput")
    b1_d = nc.dram_tensor("b1", [P, NN], F32, kind="ExternalInput")
    b2_d = nc.dram_tensor("b2", [P, NC], F32, kind="ExternalInput")
    onesh_d = nc.dram_tensor("onesh", [P, P], FP16, kind="ExternalInput")
    zer_d = nc.dram_tensor("zer64", [HD, TO], FP16, kind="ExternalInput")
    ones1_d = nc.dram_tensor("ones1", [1, P], BF16, kind="ExternalInput")
    yt_d = nc.dram_tensor("yt", [P, NC, TO], F32, kind="ExternalOutput")

    with tile.TileContext(nc) as tc:
        with (
            tc.tile_pool(name="main", bufs=1) as mp,
            tc.tile_pool(name="db", bufs=2) as db,
            tc.tile_pool(name="expp", bufs=4) as expp,
            tc.tile_pool(name="pbig", bufs=3, space="PSUM") as pbig,
            tc.tile_pool(name="po", bufs=2, space="PSUM") as po,
        ):
            # ---- inputs: xtf pieces FIRST (each dma_start costs ~1us of
            # serial TENSOR_LOAD issue on the Sync queue, so critical data
            # goes ahead of every small constant) ----
            xtf = mp.tile([P, NC, S], FP16, tag="xtf")  # x^T -> xn^T in place
            onesh = mp.tile([P, P], FP16, tag="onesh")
            # first LN job needs pieces (c, 0) plus the ones matrix
            for c in range(NC):
                nc.sync.dma_start(out=xtf[:, c, 0:512],
                                  in_=xtf_d[:, c, 0:512])
            nc.sync.dma_start(out=onesh[:], in_=onesh_d[:])
            for t0 in range(512, S, 512):
                for c in range(NC):
                    nc.sync.dma_start(out=xtf[:, c, t0:t0 + 512],
                                      in_=xtf_d[:, c, t0:t0 + 512])
            wk_t = mp.tile([P, NC, D], FP16, tag="wk")
            nc.sync.dma_start(out=wk_t[:], in_=wk_d[:])
            wv_t = mp.tile([P, NC, D], FP16, tag="wv")
            nc.sync.dma_start(out=wv_t[:], in_=wv_d[:])
            wq_t = mp.tile([P, NC, D], FP16, tag="wq")
            nc.sync.dma_start(out=wq_t[:], in_=wq_d[:])
            eps_t = mp.tile([P, 1], F32, tag="eps")
            nc.vector.memset(eps_t, EPS)
            # logit shift: exp(s-25) keeps softmax denominators within the
            # ACT Ln table's domain; numerator and denominator scale by the
            # same e^-25, so the softmax is unchanged
            sh_t = mp.tile([P, 1], F32, tag="sh")
            nc.vector.memset(sh_t, -25.0)
            ones1 = mp.tile([1, P], BF16, tag="ones1")
            nc.sync.dma_start(out=ones1[:], in_=ones1_d[:])
            bq_t = mp.tile([P, NE], F32, tag="bq")
            nc.sync.dma_start(out=bq_t[:], in_=bq_d[:])
            bp_t = mp.tile([P, NC], F32, tag="bp")
            nc.sync.dma_start(out=bp_t[:], in_=bp_d[:])
            b1_t = mp.tile([P, NN], F32, tag="b1")
            nc.sync.dma_start(out=b1_t[:], in_=b1_d[:])
            b2_t = mp.tile([P, NC], F32, tag="b2")
            nc.sync.dma_start(out=b2_t[:], in_=b2_d[:])
            xto = mp.tile([P, NC, TO], F32, tag="xto")  # raw x^T ours (residual)
            nc.sync.dma_start(out=xto[:], in_=xto_d[:])
            wp_t = mp.tile([P, NC, D], FP16, tag="wp")
            nc.sync.dma_start(out=wp_t[:], in_=wp_d[:])

            # ---- transposed layernorm ----
            # jobs: list of (src, dst, t0, idx). Stats per 512-token slice via
            # all-ones stationary matmul (mean/meansq arrive broadcast on all
            # 128 partitions). src/dst bf16 -> DVE 2x mode on the elementwise.
            def ln_f32r(jobs):
                for srct, dstt, t0, i in jobs:
                    ps = pbig.tile([P, 2, 512], F32, tag="pb", name=f"lnps{i}")
                    for c in range(NC):
                        sl = srct[:, c, t0:t0 + 512]
                        sq = db.tile([P, 512], FP16, tag="sq", name=f"lnsq{i}")
                        nc.scalar.square(sq[:], sl)
                        nc.tensor.matmul(ps[:, 0, :], onesh[:], sl,
                                         start=(c == 0), stop=(c == NC - 1))
                        nc.tensor.matmul(ps[:, 1, :], onesh[:], sq[:],
                                         start=(c == 0), stop=(c == NC - 1))
                    mb = db.tile([P, 512], FP16, tag="mb", name=f"mb{i}")
                    nc.scalar.copy(out=mb[:], in_=ps[:, 0, :])
                    rb = db.tile([P, 512], FP16, tag="rb", name=f"rb{i}")
                    nc.vector.tensor_tensor(rb[:], mb[:], mb[:], OP.mult)
                    nc.vector.tensor_tensor(rb[:], ps[:, 1, :], rb[:], OP.subtract)
                    # rstd = exp(-0.5*ln(var+eps)), in place
                    nc.scalar.activation(rb[:], rb[:], AF.Ln, bias=eps_t[:])
                    nc.scalar.activation(rb[:], rb[:], AF.Exp, scale=-0.5)
                    for c in range(NC):
                        sl_in = srct[:, c, t0:t0 + 512]
                        sl_out = dstt[:, c, t0:t0 + 512]
                        nc.vector.tensor_tensor(sl_out, sl_in, mb[:], OP.subtract)
                        nc.vector.tensor_tensor(sl_out, sl_out, rb[:], OP.mult)

            # ---- phase 1: LN1 on full sequence (in-place) ----
            ln_f32r([(xtf, xtf, t0, t0 // 512) for t0 in range(0, S, 512)])

            # ---- phase 3: QKV projections ----
            # k^T [e_all, t_full]; no bias (cancels in softmax)
            kt = mp.tile([P, NE, S], FP16, tag="kt")
            for e in range(NE):
                pss = [pbig.tile([P, 2, 512], F32, tag="pb", name=f"pss{e}_{i}")
                       for i in range(2)]
                for c in range(NC):
                    for tau in range(4):
                        nc.tensor.matmul(pss[tau // 2][:, tau % 2, :],
                                         (wk_t[:, c, e * P:(e + 1) * P]),
                                         (xtf[:, c, tau * 512:(tau + 1) * 512]),
                                         start=(c == 0), stop=(c == NC - 1))
                for tau in range(4):
                    dst = kt[:, e, tau * 512:(tau + 1) * 512]
                    src = pss[tau // 2][:, tau % 2, :]
                    if tau % 2 == 0:
                        nc.vector.tensor_copy(out=dst, in_=src)
                    else:
                        nc.scalar.copy(out=dst, in_=src)

            # v [t_full, 8 heads x (64 + ones + pad)] -- natural layout,
            # ones column makes the o-matmul also produce the softmax denom;
            # stride 66 keeps each head 4B-aligned for 2x DVE copies
            vv = mp.tile([P, NT, H, HD + 2], BF16, tag="vv")
            nc.vector.memset(vv[:, :, :, HD:HD + 1], 1.0)
            nc.vector.memset(vv[:, :, :, HD + 1:HD + 2], 0.0)
            for t in range(NT):
                ps = po.tile([P, 512], F32, tag="ps", name=f"vps{t}")
                for c in range(NC):
                    nc.tensor.matmul(ps[:],
                                     (xtf[:, c, t * P:(t + 1) * P]),
                                     (wv_t[:, c, :]),
                                     start=(c == 0), stop=(c == NC - 1))
                nc.vector.tensor_copy(
                    out=vv[:, t, :, 0:HD],
                    in_=ps[:].rearrange("p (h e) -> p h e", e=HD))

            # q^T per head, zero-padded to the full 128-partition e-sub so
            # the scores matmul contracts K=128; Q projects from the
            # normalized own-block slice of xtf
            qpA = mp.tile([P, NE, TO], FP16, tag="qpA")
            qpB = mp.tile([P, NE, TO], FP16, tag="qpB")
            for e in range(NE):
                nc.sync.dma_start(out=qpA[HD:P, e, :], in_=zer_d[:])
                nc.sync.dma_start(out=qpB[0:HD, e, :], in_=zer_d[:])
            for e in range(NE):
                ps = po.tile([P, 512], F32, tag="ps", name=f"qps{e}")
                for c in range(NC):
                    nc.tensor.matmul(ps[:],
                                     (wq_t[:, c, e * P:(e + 1) * P]),
                                     (xtf[:, c, 0:TO]),
                                     start=(c == 0), stop=(c == NC - 1))
                nc.vector.tensor_scalar_add(qpA[0:HD, e, :], ps[0:HD, :],
                                            bq_t[0:HD, e:e + 1])
                nc.vector.tensor_scalar_add(qpB[HD:P, e, :], ps[HD:P, :],
                                            bq_t[HD:P, e:e + 1])

            # prefetch W1 into the slot vacated by xn^T (tag xtf)
            w1_t = mp.tile([P, NC, DFF], FP16, tag="xtf", name="w1_t")
            nc.sync.dma_start(out=w1_t[:], in_=w1_d[:])

            # ---- phase 4: attention (scores^T, exp, o accumulate) ----
            ot = mp.tile([P, NC, TO], FP16, tag="ot")  # o^T packed [he, s]
            for e in range(NE):
                opss = []
                for hh in range(2):
                    h = 2 * e + hh
                    ops = po.tile([P, 512], F32, tag="ps", name=f"po{h}")
                    opss.append(ops)
                    for g in range(8):
                        scp = pbig.tile([P, 2, 512], F32, tag="pb",
                                        name=f"scp{h}_{g}")
                        for j in range(2):
                            k = 2 * g + j
                            nc.tensor.matmul(scp[:, j, :],
                                             (kt[:, e, k * P:(k + 1) * P]),
                                             (qpA[:, e, :] if hh == 0
                                              else qpB[:, e, :]),
                                             start=True, stop=True)
                        ex = expp.tile([P, 2, 512], BF16, tag="ex",
                                       name=f"ex{h}_{g}")
                        nc.scalar.activation(ex[:], scp[:], AF.Exp, bias=sh_t[:])
                        for j in range(2):
                            k = 2 * g + j
                            nc.tensor.matmul(ops[0:HD + 1, :],
                                             (vv[:, k, h, 0:HD + 1]),
                                             (ex[:, j, :]),
                                             start=(k == 0), stop=(k == NT - 1))
                    # evacuate numerators + denominator row to SBUF right
                    # away; the PSUM o-accumulator frees so the next head's
                    # o-matmuls never stall on this head's normalization
                    onum = db.tile([HD, 512], F32, tag="onum",
                                   name=f"onum{h}")
                    nc.vector.tensor_copy(out=onum[:], in_=ops[0:HD, :])
                    dr = db.tile([1, 512], F32, tag="dr", name=f"dr{h}")
                    nc.vector.tensor_copy(out=dr[:], in_=ops[HD:HD + 1, :])
                    # 1/d via fast DVE reciprocal (18-bit accurate), keeping
                    # ScalarE free for the attention exps
                    nc.vector.reciprocal_approx_fast(out=dr[:], in_=dr[:])
                    drb = db.tile([1, 512], BF16, tag="drb", name=f"drb{h}")
                    nc.vector.tensor_copy(out=drb[:], in_=dr[:])
                    # broadcast 1/d to 64 partitions with a K=1 ones matmul
                    rps = po.tile([P, 512], F32, tag="ps", name=f"rps{h}")
                    nc.tensor.matmul(rps[0:HD, :], ones1[:, 0:HD], drb[:],
                                     start=True, stop=True)
                    rbc = db.tile([HD, 512], F32, tag="rbc", name=f"rbc{h}")
                    nc.vector.tensor_copy(out=rbc[:], in_=rps[0:HD, :])
                    nc.vector.tensor_tensor(
                        ot[hh * HD:(hh + 1) * HD, e, :],
                        onum[:], rbc[:], OP.mult)

            # prefetch W2 into the slot vacated by k^T (tag kt)
            w2_t = mp.tile([P, NN, D], FP16, tag="kt", name="w2_t")
            nc.sync.dma_start(out=w2_t[:], in_=w2_d[:])

            # ---- phase 5: output projection + residual (transposed) ----
            xat = mp.tile([P, NC, TO], F32, tag="xat")
            for dsub in range(NC):
                ps = po.tile([P, 512], F32, tag="ps", name=f"pjps{dsub}")
                for hc in range(NC):
                    nc.tensor.matmul(ps[:],
                                     (wp_t[:, hc, dsub * P:(dsub + 1) * P]),
                                     (ot[:, hc, :]),
                                     start=(hc == 0), stop=(hc == NC - 1))
                nc.vector.scalar_tensor_tensor(
                    xat[:, dsub, :], ps[:], bp_t[:, dsub:dsub + 1],
                    xto[:, dsub, :], OP.add, OP.add)

            # ---- phase 6: LN2 (stats on a bf16 shadow of the residual) ----
            xatb = mp.tile([P, NC, TO], FP16, tag="xatb")
            for c in range(NC):
                nc.scalar.copy(out=xatb[:, c, :], in_=xat[:, c, :])
            xn2 = mp.tile([P, NC, TO], FP16, tag="xn2")
            ps2 = pbig.tile([P, 2, 512], F32, tag="pb", name="ln2ps")
            for c in range(NC):
                sq3 = db.tile([P, 512], FP16, tag="sq3", name="ln2sq")
                nc.scalar.square(sq3[:], xatb[:, c, :])
                nc.tensor.matmul(ps2[:, 0, :], onesh[:], xatb[:, c, :],
                                 start=(c == 0), stop=(c == NC - 1))
                nc.tensor.matmul(ps2[:, 1, :], onesh[:], sq3[:],
                                 start=(c == 0), stop=(c == NC - 1))
            mb2 = mp.tile([P, 512], FP16, tag="lnmb2")
            nc.scalar.copy(out=mb2[:], in_=ps2[:, 0, :])
            rb2 = mp.tile([P, 512], FP16, tag="lnrb2")
            nc.vector.tensor_tensor(rb2[:], mb2[:], mb2[:], OP.mult)
            nc.vector.tensor_tensor(rb2[:], ps2[:, 1, :], rb2[:], OP.subtract)
            nc.scalar.activation(rb2[:], rb2[:], AF.Ln, bias=eps_t[:])
            nc.scalar.activation(rb2[:], rb2[:], AF.Exp, scale=-0.5)
            for c in range(NC):
                nc.vector.tensor_tensor(xn2[:, c, :], xatb[:, c, :], mb2[:],
                                        OP.subtract)
                nc.vector.tensor_tensor(xn2[:, c, :], xn2[:, c, :], rb2[:],
                                        OP.mult)

            # ---- phase 7: MLP in -- h^T = relu(W1^T xn2 + b1) on ScalarE ----
            ht = mp.tile([P, NN, TO], FP16, tag="ht")
            for n in range(NN):
                ps = po.tile([P, 512], F32, tag="ps", name=f"h1ps{n}")
                for c in range(NC):
                    nc.tensor.matmul(ps[:],
                                     (w1_t[:, c, n * P:(n + 1) * P]),
                                     (xn2[:, c, :]),
                                     start=(c == 0), stop=(c == NC - 1))
                nc.scalar.activation(ht[:, n, :], ps[:], AF.Relu,
                                     bias=b1_t[:, n:n + 1])

            # ---- phase 8: MLP out + residual, y^T [d, t] ----
            # yt reuses xto's slot (residual dead after the proj add)
            yt = mp.tile([P, NC, TO], F32, tag="xto", name="yt")
            for dsub in range(NC):
                ps = po.tile([P, 512], F32, tag="ps", name=f"h2ps{dsub}")
                for n in range(NN):
                    nc.tensor.matmul(ps[:],
                                     (w2_t[:, n, dsub * P:(dsub + 1) * P]),
                                     (ht[:, n, :]),
                                     start=(n == 0), stop=(n == NN - 1))
                nc.vector.scalar_tensor_tensor(
                    yt[:, dsub, :], ps[:], b2_t[:, dsub:dsub + 1],
                    xat[:, dsub, :], OP.add, OP.add)
                nc.sync.dma_start(out=yt_d[:, dsub, :], in_=yt[:, dsub, :])

    return nc


def _chunk_p(a):
    """[K, N] -> [128, K//128, N] (partition-major SBUF layout)."""
    K = a.shape[0]
    return np.ascontiguousarray(
        a.reshape(K // P, P, *a.shape[1:]).transpose(1, 0, *range(2, a.ndim + 1)))


def host_inputs(x, Wq, bq, Wk, bk, Wv, bv, Wp, bp, W1, b1, W2, b2,
                g1, be1, g2, be2):
    """Fold LN affines into the projections, pre-transpose/chunk everything.

    bk is dropped (constant-per-query score shift cancels in softmax).
    bv is folded into bp: o = o_nodiv/denom + bv, so attn = o@Wp + bp
    becomes o_div@Wp + (bp + bv@Wp).
    """
    f = np.float32
    Wq_all = np.ascontiguousarray(Wq.transpose(1, 0, 2).reshape(D, D), f)
    Wk_all = np.ascontiguousarray(Wk.transpose(1, 0, 2).reshape(D, D), f)
    Wv_all = np.ascontiguousarray(Wv.transpose(1, 0, 2).reshape(D, D), f)
    bq_all = (bq.reshape(D) + be1 @ Wq_all).astype(f)
    bv_all = (bv.reshape(D) + be1 @ Wv_all).astype(f)
    Wq_f = Wq_all * g1[:, None]
    Wk_f = Wk_all * g1[:, None]
    Wv_f = Wv_all * g1[:, None]
    W1_f = (W1 * g2[:, None]).astype(f)
    b1_f = (b1 + be2 @ W1).astype(f)
    bp_f = (bp + bv_all @ Wp).astype(f)

    shared = {
        "wq": _chunk_p(Wq_f.astype(f)).astype(F16),
        "wk": _chunk_p(Wk_f.astype(f)).astype(F16),
        "wv": _chunk_p(Wv_f.astype(f)).astype(F16),
        "wp": _chunk_p(Wp.astype(f)).astype(F16),
        "w1": _chunk_p(W1_f).astype(F16),
        "w2": _chunk_p(W2.astype(f)).astype(F16),
        "bq": np.ascontiguousarray(bq_all.reshape(NE, P).T),
        "bp": np.ascontiguousarray(bp_f.reshape(NC, P).T),
        "b1": np.ascontiguousarray(b1_f.reshape(NN, P).T),
        "b2": np.ascontiguousarray(b2.astype(f).reshape(NC, P).T),
        "onesh": np.full((P, P), 1.0 / D, F16),
        "zer64": np.zeros((HD, TO), F16),
        "ones1": np.ones((1, P), BF),
    }
    in_maps = []
    for c in range(8):
        b, qb = c // 4, c % 4
        # roll so each core's own query block sits first: Q projection
        # always reads xtf[:, :, 0:TO]
        xT = np.ascontiguousarray(
            np.roll(x[b].T.astype(f), -qb * TO, axis=1))    # [D, S]
        m = dict(shared)
        m["xtf"] = _chunk_p(xT).astype(F16)
        m["xto"] = _chunk_p(np.ascontiguousarray(xT[:, 0:TO]))
        in_maps.append(m)
    return in_maps


def assemble_output(results, dtype):
    y = np.empty((B, S, D), np.float32)
    for c in range(8):
        b, qb = c // 4, c % 4
        yt = results[c]["yt"]                                 # [P, NC, TO]
        yT = yt.transpose(1, 0, 2).reshape(D, TO)             # [D, TO]
        y[b, qb * TO:(qb + 1) * TO, :] = yT.T
    return y.astype(dtype, copy=False)


def _enable_ntff_trace():
    """The image's `antenv` lacks `axon_hooks`; inject it and register the
    ctypes NTFF profile hook from trn_boot so trace=True yields exec times."""
    import types
    if "antenv.axon_hooks" not in sys.modules:
        mod = types.ModuleType("antenv.axon_hooks")
        mod._hook = None
        mod.set_axon_ntff_profile_hook = lambda h: setattr(mod, "_hook", h)
        mod.get_axon_ntff_profile_hook = lambda: mod._hook
        sys.modules["antenv.axon_hooks"] = mod
    import antenv.axon_hooks as ah
    if ah.get_axon_ntff_profile_hook() is None:
        try:
            from trn_agent_boot.trn_boot import _ntff_profile_via_ctypes
            ah.set_axon_ntff_profile_hook(
                _ntff_profile_via_ctypes("/opt/axon/libaxon_pjrt.so"))
        except Exception:
            pass
    import concourse.bass_utils as bu
    bu.upload_artifacts = lambda d: d  # no artifact bucket in this container


def kernel(**inputs) -> np.ndarray:
    global LAST_EXEC_NS
    in_maps = host_inputs(**{k: np.asarray(v) for k, v in inputs.items()})
    nc = build_program()
    nc.finalize()
    trace = os.environ.get("KERNEL_TRACE", "0") == "1"
    kwargs = {}
    if trace:
        _enable_ntff_trace()
        tmpdir = os.environ.get("KERNEL_TRACE_DIR", "/tmp/ktrace")
        os.makedirs(tmpdir, exist_ok=True)
        kwargs["tmpdir"] = tmpdir
    res = run_bass_kernel_spmd(nc, in_maps, list(range(8)), trace=trace, **kwargs)
    LAST_EXEC_NS = res.exec_time_ns
    return assemble_output(res.results, np.asarray(inputs["x"]).dtype)


# revision 32
# speedup vs baseline: 1.2207x; 1.0012x over previous
"""Fused transformer block (pre-norm attention + MLP) for Trainium2, 8 NeuronCores.

Problem: B=2, S=2048, D=512, H=8, HD=64, fp32 in/out.

Sharding: pure data-parallel over (batch, query-block). Core c handles batch
b = c//4 and query rows [qb*512, (qb+1)*512) with qb = c%4. Each core
redundantly computes LN1 + K/V for the full 2048-token sequence of its batch
element (no collectives needed).

v2 changes vs baseline:
- bf16 datapath for every matmul (weights pre-cast on host; activations
  written bf16 at PSUM evacuation). PSUM accumulation stays fp32. Residual
  stream (xto/xat/yt) stays fp32.
- K bias dropped entirely (constant-per-query score shift cancels in
  softmax); V bias folded into the proj bias on host (bp' = bp + bv@Wp).
- The redundant own-block re-normalization (xno) removed; Q projects from
  the already-normalized xtf slice.
- MLP ReLU+bias moved to ScalarE (Relu is in the natural_log_exp table set,
  so no table reload); frees the Vector engine.
- LN math in bf16 2x DVE mode where operands live in SBUF.
- LN2 stats use f32r bitcast views of the fp32 residual (no cast pass).
"""

import os
import sys

sys.path.insert(0, "/opt/trn_rl_repo")

import numpy as np
import ml_dtypes

import concourse.bass as bass
import concourse.mybir as mybir
import concourse.tile as tile
from concourse import bacc
from concourse.bass_utils import run_bass_kernel_spmd

P = 128
B, S, D, H = 2, 2048, 512, 8
HD = D // H          # 64
DFF = 4 * D          # 2048
TO = 512             # tokens owned per core (query block)
NC = D // P          # 4 d-chunks
NE = D // P          # 4 e_all-subs (q/k head pairs)
NT = S // P          # 16 t-subs
NN = DFF // P        # 16 n-subs
EPS = 1e-5
F32 = mybir.dt.float32
F32R = mybir.dt.float32r
BF16 = mybir.dt.bfloat16
AF = mybir.ActivationFunctionType
OP = mybir.AluOpType
BF = ml_dtypes.bfloat16
FP16 = mybir.dt.float16
F16 = np.float16

LAST_EXEC_NS = None

# Force every Exp/Ln ACTIVATE onto the one table set containing both, so the
# ACT table never reloads mid-kernel (each reload is ~2.7us and serializes
# against the attention exps). Copy/Identity/Relu/Square are in every set.
_orig_gat = bacc.get_activation_tables


def _gat_nlexp_only(arch):
    tabs = _orig_gat(arch)
    for name, fns in tabs.items():
        if name != "natural_log_exp_and_others":
            fns.discard(AF.Exp)
            fns.discard(AF.Ln)
    return tabs


bacc.get_activation_tables = _gat_nlexp_only

def build_program():
    nc = bacc.Bacc("TRN2", target_bir_lowering=False, debug=False, num_devices=8)

    xtf_d = nc.dram_tensor("xtf", [P, NC, S], FP16, kind="ExternalInput")
    xto_d = nc.dram_tensor("xto", [P, NC, TO], F32, kind="ExternalInput")
    wq_d = nc.dram_tensor("wq", [P, NC, D], FP16, kind="ExternalInput")
    wk_d = nc.dram_tensor("wk", [P, NC, D], FP16, kind="ExternalInput")
    wv_d = nc.dram_tensor("wv", [P, NC, D], FP16, kind="ExternalInput")
    wp_d = nc.dram_tensor("wp", [P, NC, D], FP16, kind="ExternalInput")
    w1_d = nc.dram_tensor("w1", [P, NC, DFF], FP16, kind="ExternalInput")
    w2_d = nc.dram_tensor("w2", [P, NN, D], FP16, kind="ExternalInput")
    bq_d = nc.dram_tensor("bq", [P, NE], F32, kind="ExternalInput")
    bp_d = nc.dram_tensor("bp", [P, NC], F32, kind="ExternalInput")
    b1_d = nc.dram_tensor("b1", [P, NN], F32, kind="ExternalInput")
    b2_d = nc.dram_tensor("b2", [P, NC], F32, kind="ExternalInput")
    onesh_d = nc.dram_tensor("onesh", [P, P], FP16, kind="ExternalInput")
    zer_d = nc.dram_tensor("zer64", [HD, TO], FP16, kind="ExternalInput")
    ones1_d = nc.dram_tensor("ones1", [1, P], BF16, kind="ExternalInput")
    yt_d = nc.dram_tensor("yt", [P, NC, TO], F32, kind="ExternalOutput")

    with tile.TileContext(nc) as tc:
        with (
            tc.tile_pool(name="main", bufs=1) as mp,
            tc.tile_pool(name="db", bufs=2) as db,
            tc.tile_pool(name="expp", bufs=4) as expp,
            tc.tile_pool(name="pbig", bufs=3, space="PSUM") as pbig,
            tc.tile_pool(name="po", bufs=2, space="PSUM") as po,
        ):
            # ---- inputs: xtf pieces FIRST (each dma_start costs ~1us of
            # serial TENSOR_LOAD issue on the Sync queue, so critical data
            # goes ahead of every small constant) ----
            xtf = mp.tile([P, NC, S], FP16, tag="xtf")  # x^T -> xn^T in place
            onesh = mp.tile([P, P], FP16, tag="onesh")
            # the first stats matmul needs only piece (0, 0) and the
            # ones matrix -- put exactly those two first so LN compute
            # starts after 2 serial TENSOR_LOAD issues instead of 5
            nc.sync.dma_start(out=xtf[:, 0, 0:512], in_=xtf_d[:, 0, 0:512])
            nc.sync.dma_start(out=onesh[:], in_=onesh_d[:])
            for c in range(1, NC):
                nc.sync.dma_start(out=xtf[:, c, 0:512],
                                  in_=xtf_d[:, c, 0:512])
            for t0 in range(512, S, 512):
                for c in range(NC):
                    nc.sync.dma_start(out=xtf[:, c, t0:t0 + 512],
                                      in_=xtf_d[:, c, t0:t0 + 512])
            wk_t = mp.tile([P, NC, D], FP16, tag="wk")
            nc.sync.dma_start(out=wk_t[:], in_=wk_d[:])
            wv_t = mp.tile([P, NC, D], FP16, tag="wv")
            nc.sync.dma_start(out=wv_t[:], in_=wv_d[:])
            wq_t = mp.tile([P, NC, D], FP16, tag="wq")
            nc.sync.dma_start(out=wq_t[:], in_=wq_d[:])
            eps_t = mp.tile([P, 1], F32, tag="eps")
            nc.vector.memset(eps_t, EPS)
            # logit shift: exp(s-25) keeps softmax denominators within the
            # ACT Ln table's domain; numerator and denominator scale by the
            # same e^-25, so the softmax is unchanged
            sh_t = mp.tile([P, 1], F32, tag="sh")
            nc.vector.memset(sh_t, -25.0)
            ones1 = mp.tile([1, P], BF16, tag="ones1")
            nc.sync.dma_start(out=ones1[:], in_=ones1_d[:])
            bq_t = mp.tile([P, NE], F32, tag="bq")
            nc.sync.dma_start(out=bq_t[:], in_=bq_d[:])
            bp_t = mp.tile([P, NC], F32, tag="bp")
            nc.sync.dma_start(out=bp_t[:], in_=bp_d[:])
            b1_t = mp.tile([P, NN], F32, tag="b1")
            nc.sync.dma_start(out=b1_t[:], in_=b1_d[:])
            b2_t = mp.tile([P, NC], F32, tag="b2")
            nc.sync.dma_start(out=b2_t[:], in_=b2_d[:])
            xto = mp.tile([P, NC, TO], F32, tag="xto")  # raw x^T ours (residual)
            nc.sync.dma_start(out=xto[:], in_=xto_d[:])
            wp_t = mp.tile([P, NC, D], FP16, tag="wp")
            nc.sync.dma_start(out=wp_t[:], in_=wp_d[:])

            # ---- transposed layernorm ----
            # jobs: list of (src, dst, t0, idx). Stats per 512-token slice via
            # all-ones stationary matmul (mean/meansq arrive broadcast on all
            # 128 partitions). src/dst bf16 -> DVE 2x mode on the elementwise.
            def ln_f32r(jobs):
                for srct, dstt, t0, i in jobs:
                    ps = pbig.tile([P, 2, 512], F32, tag="pb", name=f"lnps{i}")
                    for c in range(NC):
                        sl = srct[:, c, t0:t0 + 512]
                        sq = db.tile([P, 512], FP16, tag="sq", name=f"lnsq{i}")
                        nc.scalar.square(sq[:], sl)
                        nc.tensor.matmul(ps[:, 0, :], onesh[:], sl,
                                         start=(c == 0), stop=(c == NC - 1))
                        nc.tensor.matmul(ps[:, 1, :], onesh[:], sq[:],
                                         start=(c == 0), stop=(c == NC - 1))
                    mb = db.tile([P, 512], FP16, tag="mb", name=f"mb{i}")
                    nc.scalar.copy(out=mb[:], in_=ps[:, 0, :])
                    rb = db.tile([P, 512], FP16, tag="rb", name=f"rb{i}")
                    nc.vector.tensor_tensor(rb[:], mb[:], mb[:], OP.mult)
                    nc.vector.tensor_tensor(rb[:], ps[:, 1, :], rb[:], OP.subtract)
                    # rstd = exp(-0.5*ln(var+eps)), in place
                    nc.scalar.activation(rb[:], rb[:], AF.Ln, bias=eps_t[:])
                    nc.scalar.activation(rb[:], rb[:], AF.Exp, scale=-0.5)
                    for c in range(NC):
                        sl_in = srct[:, c, t0:t0 + 512]
                        sl_out = dstt[:, c, t0:t0 + 512]
                        nc.vector.tensor_tensor(sl_out, sl_in, mb[:], OP.subtract)
                        nc.vector.tensor_tensor(sl_out, sl_out, rb[:], OP.mult)

            # ---- phase 1: LN1 on full sequence (in-place) ----
            ln_f32r([(xtf, xtf, t0, t0 // 512) for t0 in range(0, S, 512)])

            # ---- phase 3: QKV projections ----
            # k^T [e_all, t_full]; no bias (cancels in softmax)
            kt = mp.tile([P, NE, S], FP16, tag="kt")
            for e in range(NE):
                pss = [pbig.tile([P, 2, 512], F32, tag="pb", name=f"pss{e}_{i}")
                       for i in range(2)]
                for c in range(NC):
                    for tau in range(4):
                        nc.tensor.matmul(pss[tau // 2][:, tau % 2, :],
                                         (wk_t[:, c, e * P:(e + 1) * P]),
                                         (xtf[:, c, tau * 512:(tau + 1) * 512]),
                                         start=(c == 0), stop=(c == NC - 1))
                for tau in range(4):
                    dst = kt[:, e, tau * 512:(tau + 1) * 512]
                    src = pss[tau // 2][:, tau % 2, :]
                    if tau % 2 == 0:
                        nc.vector.tensor_copy(out=dst, in_=src)
                    else:
                        nc.scalar.copy(out=dst, in_=src)

            # v [t_full, 8 heads x (64 + ones + pad)] -- natural layout,
            # ones column makes the o-matmul also produce the softmax denom;
            # stride 66 keeps each head 4B-aligned for 2x DVE copies
            vv = mp.tile([P, NT, H, HD + 2], BF16, tag="vv")
            nc.vector.memset(vv[:, :, :, HD:HD + 1], 1.0)
            nc.vector.memset(vv[:, :, :, HD + 1:HD + 2], 0.0)
            for t in range(NT):
                ps = po.tile([P, 512], F32, tag="ps", name=f"vps{t}")
                for c in range(NC):
                    nc.tensor.matmul(ps[:],
                                     (xtf[:, c, t * P:(t + 1) * P]),
                                     (wv_t[:, c, :]),
                                     start=(c == 0), stop=(c == NC - 1))
                nc.vector.tensor_copy(
                    out=vv[:, t, :, 0:HD],
                    in_=ps[:].rearrange("p (h e) -> p h e", e=HD))

            # q^T per head, zero-padded to the full 128-partition e-sub so
            # the scores matmul contracts K=128; Q projects from the
            # normalized own-block slice of xtf
            qpA = mp.tile([P, NE, TO], FP16, tag="qpA")
            qpB = mp.tile([P, NE, TO], FP16, tag="qpB")
            for e in range(NE):
                nc.sync.dma_start(out=qpA[HD:P, e, :], in_=zer_d[:])
                nc.sync.dma_start(out=qpB[0:HD, e, :], in_=zer_d[:])
            for e in range(NE):
                ps = po.tile([P, 512], F32, tag="ps", name=f"qps{e}")
                for c in range(NC):
                    nc.tensor.matmul(ps[:],
                                     (wq_t[:, c, e * P:(e + 1) * P]),
                                     (xtf[:, c, 0:TO]),
                                     start=(c == 0), stop=(c == NC - 1))
                nc.vector.tensor_scalar_add(qpA[0:HD, e, :], ps[0:HD, :],
                                            bq_t[0:HD, e:e + 1])
                nc.vector.tensor_scalar_add(qpB[HD:P, e, :], ps[HD:P, :],
                                            bq_t[HD:P, e:e + 1])

            # prefetch W1 into the slot vacated by xn^T (tag xtf)
            w1_t = mp.tile([P, NC, DFF], FP16, tag="xtf", name="w1_t")
            nc.sync.dma_start(out=w1_t[:], in_=w1_d[:])

            # ---- phase 4: attention (scores^T, exp, o accumulate) ----
            ot = mp.tile([P, NC, TO], FP16, tag="ot")  # o^T packed [he, s]
            for e in range(NE):
                opss = []
                for hh in range(2):
                    h = 2 * e + hh
                    ops = po.tile([P, 512], F32, tag="ps", name=f"po{h}")
                    opss.append(ops)
                    for g in range(8):
                        scp = pbig.tile([P, 2, 512], F32, tag="pb",
                                        name=f"scp{h}_{g}")
                        for j in range(2):
                            k = 2 * g + j
                            nc.tensor.matmul(scp[:, j, :],
                                             (kt[:, e, k * P:(k + 1) * P]),
                                             (qpA[:, e, :] if hh == 0
                                              else qpB[:, e, :]),
                                             start=True, stop=True)
                        ex = expp.tile([P, 2, 512], BF16, tag="ex",
                                       name=f"ex{h}_{g}")
                        nc.scalar.activation(ex[:], scp[:], AF.Exp, bias=sh_t[:])
                        for j in range(2):
                            k = 2 * g + j
                            nc.tensor.matmul(ops[0:HD + 1, :],
                                             (vv[:, k, h, 0:HD + 1]),
                                             (ex[:, j, :]),
                                             start=(k == 0), stop=(k == NT - 1))
                    # evacuate numerators + denominator row to SBUF right
                    # away; the PSUM o-accumulator frees so the next head's
                    # o-matmuls never stall on this head's normalization
                    onum = db.tile([HD, 512], F32, tag="onum",
                                   name=f"onum{h}")
                    nc.vector.tensor_copy(out=onum[:], in_=ops[0:HD, :])
                    dr = db.tile([1, 512], F32, tag="dr", name=f"dr{h}")
                    nc.vector.tensor_copy(out=dr[:], in_=ops[HD:HD + 1, :])
                    # 1/d via fast DVE reciprocal (18-bit accurate), keeping
                    # ScalarE free for the attention exps
                    nc.vector.reciprocal_approx_fast(out=dr[:], in_=dr[:])
                    drb = db.tile([1, 512], BF16, tag="drb", name=f"drb{h}")
                    nc.vector.tensor_copy(out=drb[:], in_=dr[:])
                    # broadcast 1/d to 64 partitions with a K=1 ones matmul
                    rps = po.tile([P, 512], F32, tag="ps", name=f"rps{h}")
                    nc.tensor.matmul(rps[0:HD, :], ones1[:, 0:HD], drb[:],
                                     start=True, stop=True)
                    rbc = db.tile([HD, 512], F32, tag="rbc", name=f"rbc{h}")
                    nc.vector.tensor_copy(out=rbc[:], in_=rps[0:HD, :])
                    nc.vector.tensor_tensor(
                        ot[hh * HD:(hh + 1) * HD, e, :],
                        onum[:], rbc[:], OP.mult)

            # prefetch W2 into the slot vacated by k^T (tag kt)
            w2_t = mp.tile([P, NN, D], FP16, tag="kt", name="w2_t")
            nc.sync.dma_start(out=w2_t[:], in_=w2_d[:])

            # ---- phase 5: output projection + residual (transposed) ----
            xat = mp.tile([P, NC, TO], F32, tag="xat")
            for dsub in range(NC):
                ps = po.tile([P, 512], F32, tag="ps", name=f"pjps{dsub}")
                for hc in range(NC):
                    nc.tensor.matmul(ps[:],
                                     (wp_t[:, hc, dsub * P:(dsub + 1) * P]),
                                     (ot[:, hc, :]),
                                     start=(hc == 0), stop=(hc == NC - 1))
                nc.vector.scalar_tensor_tensor(
                    xat[:, dsub, :], ps[:], bp_t[:, dsub:dsub + 1],
                    xto[:, dsub, :], OP.add, OP.add)

            # ---- phase 6: LN2 (stats on a bf16 shadow of the residual) ----
            xatb = mp.tile([P, NC, TO], FP16, tag="xatb")
            for c in range(NC):
                nc.scalar.copy(out=xatb[:, c, :], in_=xat[:, c, :])
            xn2 = mp.tile([P, NC, TO], FP16, tag="xn2")
            ps2 = pbig.tile([P, 2, 512], F32, tag="pb", name="ln2ps")
            for c in range(NC):
                sq3 = db.tile([P, 512], FP16, tag="sq3", name="ln2sq")
                nc.scalar.square(sq3[:], xatb[:, c, :])
                nc.tensor.matmul(ps2[:, 0, :], onesh[:], xatb[:, c, :],
                                 start=(c == 0), stop=(c == NC - 1))
                nc.tensor.matmul(ps2[:, 1, :], onesh[:], sq3[:],
                                 start=(c == 0), stop=(c == NC - 1))
            mb2 = mp.tile([P, 512], FP16, tag="lnmb2")
            nc.scalar.copy(out=mb2[:], in_=ps2[:, 0, :])
            rb2 = mp.tile([P, 512], FP16, tag="lnrb2")
            nc.vector.tensor_tensor(rb2[:], mb2[:], mb2[:], OP.mult)
            nc.vector.tensor_tensor(rb2[:], ps2[:, 1, :], rb2[:], OP.subtract)
            nc.scalar.activation(rb2[:], rb2[:], AF.Ln, bias=eps_t[:])
            nc.scalar.activation(rb2[:], rb2[:], AF.Exp, scale=-0.5)
            for c in range(NC):
                nc.vector.tensor_tensor(xn2[:, c, :], xatb[:, c, :], mb2[:],
                                        OP.subtract)
                nc.vector.tensor_tensor(xn2[:, c, :], xn2[:, c, :], rb2[:],
                                        OP.mult)

            # ---- phase 7: MLP in -- h^T = relu(W1^T xn2 + b1) on ScalarE ----
            ht = mp.tile([P, NN, TO], FP16, tag="ht")
            for n in range(NN):
                ps = po.tile([P, 512], F32, tag="ps", name=f"h1ps{n}")
                for c in range(NC):
                    nc.tensor.matmul(ps[:],
                                     (w1_t[:, c, n * P:(n + 1) * P]),
                                     (xn2[:, c, :]),
                                     start=(c == 0), stop=(c == NC - 1))
                nc.scalar.activation(ht[:, n, :], ps[:], AF.Relu,
                                     bias=b1_t[:, n:n + 1])

            # ---- phase 8: MLP out + residual, y^T [d, t] ----
            # yt reuses xto's slot (residual dead after the proj add)
            yt = mp.tile([P, NC, TO], F32, tag="xto", name="yt")
            for dsub in range(NC):
                ps = po.tile([P, 512], F32, tag="ps", name=f"h2ps{dsub}")
                for n in range(NN):
                    nc.tensor.matmul(ps[:],
                                     (w2_t[:, n, dsub * P:(dsub + 1) * P]),
                                     (ht[:, n, :]),
                                     start=(n == 0), stop=(n == NN - 1))
                nc.vector.scalar_tensor_tensor(
                    yt[:, dsub, :], ps[:], b2_t[:, dsub:dsub + 1],
                    xat[:, dsub, :], OP.add, OP.add)
                nc.sync.dma_start(out=yt_d[:, dsub, :], in_=yt[:, dsub, :])

    return nc


def _chunk_p(a):
    """[K, N] -> [128, K//128, N] (partition-major SBUF layout)."""
    K = a.shape[0]
    return np.ascontiguousarray(
        a.reshape(K // P, P, *a.shape[1:]).transpose(1, 0, *range(2, a.ndim + 1)))


def host_inputs(x, Wq, bq, Wk, bk, Wv, bv, Wp, bp, W1, b1, W2, b2,
                g1, be1, g2, be2):
    """Fold LN affines into the projections, pre-transpose/chunk everything.

    bk is dropped (constant-per-query score shift cancels in softmax).
    bv is folded into bp: o = o_nodiv/denom + bv, so attn = o@Wp + bp
    becomes o_div@Wp + (bp + bv@Wp).
    """
    f = np.float32
    Wq_all = np.ascontiguousarray(Wq.transpose(1, 0, 2).reshape(D, D), f)
    Wk_all = np.ascontiguousarray(Wk.transpose(1, 0, 2).reshape(D, D), f)
    Wv_all = np.ascontiguousarray(Wv.transpose(1, 0, 2).reshape(D, D), f)
    bq_all = (bq.reshape(D) + be1 @ Wq_all).astype(f)
    bv_all = (bv.reshape(D) + be1 @ Wv_all).astype(f)
    Wq_f = Wq_all * g1[:, None]
    Wk_f = Wk_all * g1[:, None]
    Wv_f = Wv_all * g1[:, None]
    W1_f = (W1 * g2[:, None]).astype(f)
    b1_f = (b1 + be2 @ W1).astype(f)
    bp_f = (bp + bv_all @ Wp).astype(f)

    shared = {
        "wq": _chunk_p(Wq_f.astype(f)).astype(F16),
        "wk": _chunk_p(Wk_f.astype(f)).astype(F16),
        "wv": _chunk_p(Wv_f.astype(f)).astype(F16),
        "wp": _chunk_p(Wp.astype(f)).astype(F16),
        "w1": _chunk_p(W1_f).astype(F16),
        "w2": _chunk_p(W2.astype(f)).astype(F16),
        "bq": np.ascontiguousarray(bq_all.reshape(NE, P).T),
        "bp": np.ascontiguousarray(bp_f.reshape(NC, P).T),
        "b1": np.ascontiguousarray(b1_f.reshape(NN, P).T),
        "b2": np.ascontiguousarray(b2.astype(f).reshape(NC, P).T),
        "onesh": np.full((P, P), 1.0 / D, F16),
        "zer64": np.zeros((HD, TO), F16),
        "ones1": np.ones((1, P), BF),
    }
    in_maps = []
    for c in range(8):
        b, qb = c // 4, c % 4
        # roll so each core's own query block sits first: Q projection
        # always reads xtf[:, :, 0:TO]
        xT = np.ascontiguousarray(
            np.roll(x[b].T.astype(f), -qb * TO, axis=1))    # [D, S]
        m = dict(shared)
        m["xtf"] = _chunk_p(xT).astype(F16)
        m["xto"] = _chunk_p(np.ascontiguousarray(xT[:, 0:TO]))
        in_maps.append(m)
    return in_maps


def assemble_output(results, dtype):
    y = np.empty((B, S, D), np.float32)
    for c in range(8):
        b, qb = c // 4, c % 4
        yt = results[c]["yt"]                                 # [P, NC, TO]
        yT = yt.transpose(1, 0, 2).reshape(D, TO)             # [D, TO]
        y[b, qb * TO:(qb + 1) * TO, :] = yT.T
    return y.astype(dtype, copy=False)


def _enable_ntff_trace():
    """The image's `antenv` lacks `axon_hooks`; inject it and register the
    ctypes NTFF profile hook from trn_boot so trace=True yields exec times."""
    import types
    if "antenv.axon_hooks" not in sys.modules:
        mod = types.ModuleType("antenv.axon_hooks")
        mod._hook = None
        mod.set_axon_ntff_profile_hook = lambda h: setattr(mod, "_hook", h)
        mod.get_axon_ntff_profile_hook = lambda: mod._hook
        sys.modules["antenv.axon_hooks"] = mod
    import antenv.axon_hooks as ah
    if ah.get_axon_ntff_profile_hook() is None:
        try:
            from trn_agent_boot.trn_boot import _ntff_profile_via_ctypes
            ah.set_axon_ntff_profile_hook(
                _ntff_profile_via_ctypes("/opt/axon/libaxon_pjrt.so"))
        except Exception:
            pass
    import concourse.bass_utils as bu
    bu.upload_artifacts = lambda d: d  # no artifact bucket in this container


def kernel(**inputs) -> np.ndarray:
    global LAST_EXEC_NS
    in_maps = host_inputs(**{k: np.asarray(v) for k, v in inputs.items()})
    nc = build_program()
    nc.finalize()
    trace = os.environ.get("KERNEL_TRACE", "0") == "1"
    kwargs = {}
    if trace:
        _enable_ntff_trace()
        tmpdir = os.environ.get("KERNEL_TRACE_DIR", "/tmp/ktrace")
        os.makedirs(tmpdir, exist_ok=True)
        kwargs["tmpdir"] = tmpdir
    res = run_bass_kernel_spmd(nc, in_maps, list(range(8)), trace=trace, **kwargs)
    LAST_EXEC_NS = res.exec_time_ns
    return assemble_output(res.results, np.asarray(inputs["x"]).dtype)
